# revision 3
# baseline (speedup 1.0000x reference)
"""Trainium2 Bass kernel for BotanHadamardTransform: y = x @ H, with
x [4, 4096, 4096] f32 and H [4096, 4096] f32 the normalized Sylvester
Hadamard matrix H_4096 / 64.

Algorithm: Sylvester Hadamard matrices factor as Kronecker products,
H_4096 = H_A (x) H_B with A*B = 4096. For a row vector v (len 4096),
v @ H_4096 = FWHT_A applied across the A axis of (v.reshape(A, B) @ H_B).
This reduces per-row work from O(n^2) to O(n*(B + log2 A)).

Mapping to hardware (per core, 1/8 of the 16384 rows = 2048 rows):
  - host pre-transposes x so the device sees xT [4096, 2048] with the
    contraction dim on partitions (natural matmul layout, no on-device
    transposes)
  - PE contracts the low B=512 of each k-index against Hf = H[0:512,0:512]
    (which equals H_512/64 exactly) as fp32r matmuls
  - the high A=8 factor is a 3-stage FWHT butterfly across 128-partition
    chunks, elementwise adds/subs split between VectorE and GpSimdE by
    column range
  - output is written transposed (yT [4096, 2048]); host transposes back
"""
import os
import sys

sys.path.insert(0, "/opt/trn_rl_repo")

import numpy as np

import concourse.bass as bass  # noqa: F401
import concourse.tile as tile
from concourse import bacc, mybir
from concourse.bass_utils import run_bass_kernel_spmd

N_CORES = 8
N = 4096            # hidden dim
ROWS = 4 * 4096     # total rows
RC = ROWS // N_CORES  # rows (columns of xT) per core = 2048

# tunables (env-overridable for experiments)
B = int(os.environ.get("BOTAN_B", "512"))        # PE-contracted factor
R = int(os.environ.get("BOTAN_R", "256"))        # moving columns per r-tile
MM_DTYPE = os.environ.get("BOTAN_DT", "f32r")    # f32 | f32r
# columns (of R) that DVE takes per butterfly op; rest go to GpSimd
RD = int(os.environ.get("BOTAN_RD", "176"))

A = N // B               # butterfly factor
SUB = B // 128           # accumulating matmuls per output chunk
NCH = N // 128           # 32 chunks of 128 partitions
N_STAGES = A.bit_length() - 1


def _build():
    nc = bacc.Bacc("TRN2", target_bir_lowering=False, debug=False,
                   num_devices=N_CORES)
    xT_ap = nc.dram_tensor("xT", [N, RC], mybir.dt.float32,
                           kind="ExternalInput").ap()
    hf_ap = nc.dram_tensor("Hf", [B, B], mybir.dt.float32,
                           kind="ExternalInput").ap()
    yT_ap = nc.dram_tensor("yT", [N, RC], mybir.dt.float32,
                           kind="ExternalOutput").ap()

    f32 = mybir.dt.float32
    f32r = mybir.dt.float32r
    mm_dt = f32r if MM_DTYPE == "f32r" else f32

    xT_v = xT_ap.rearrange("(c p) r -> p c r", p=128)   # [128, NCH, RC]
    yT_v = yT_ap.rearrange("(c p) r -> p c r", p=128)

    n_rt = RC // R
    psum_banks_per_group = max(1, (SUB * R * 4) // 2048)
    psum_bufs = max(2, 8 // psum_banks_per_group)

    with tile.TileContext(nc) as tc:
        with (
            tc.tile_pool(name="hfp", bufs=1) as hfp,
            tc.tile_pool(name="xin", bufs=2) as xinp,
            tc.tile_pool(name="xr", bufs=4) as xrp,
            tc.tile_pool(name="w0", bufs=2) as w0p,
            tc.tile_pool(name="ps", bufs=psum_bufs, space="PSUM") as psp,
        ):
            # stationary: Hf [B, B] stored as SUB row-blocks side by side:
            # hf[p, s*B + col] = Hf[s*128 + p, col]
            hf_f32 = hfp.tile([128, SUB * B], f32, tag="hf32")
            for s in range(SUB):
                nc.sync.dma_start(hf_f32[:, s * B:(s + 1) * B],
                                  hf_ap[s * 128:(s + 1) * 128, :])
            if mm_dt == f32r:
                hf_mm = hfp.tile([128, SUB * B], f32r, tag="hfr")
                nc.scalar.copy(hf_mm[:], hf_f32[:])
            else:
                hf_mm = hf_f32

            def hf_block(s, q):
                # lhsT block [k=128 (i2 sub s), m=128 (j2 sub q)]
                return hf_mm[:, s * B + q * 128: s * B + q * 128 + 128]

            for it in range(n_rt):
                r0 = it * R
                xin = xinp.tile([128, NCH, R], f32, tag="xin")
                nc.sync.dma_start(xin[:], xT_v[:, :, r0:r0 + R])

                w0 = w0p.tile([128, NCH, R], f32, tag="w0")

                # PE transform: for each i1 group, out chunk (i1, q) =
                # sum_s HfT[s,q] @ x chunk (i1, s)
                for i1 in range(A):
                    if mm_dt == f32r:
                        # per-group rounding pass (ScalarE): f32 -> f32r
                        xg = xrp.tile([128, SUB, R], f32r, tag="xr")
                        nc.scalar.copy(xg[:],
                                       xin[:, i1 * SUB:(i1 + 1) * SUB, :])
                    else:
                        xg = xin[:, i1 * SUB:(i1 + 1) * SUB, :]
                    pg = psp.tile([128, SUB * R], f32, tag="pg")
                    for q in range(SUB):
                        for s in range(SUB):
                            nc.tensor.matmul(
                                pg[:, q * R:(q + 1) * R],
                                hf_block(s, q),
                                xg[:, s, :],
                                start=(s == 0),
                                stop=(s == SUB - 1),
                            )
                    # evict group (chunks i1*SUB .. i1*SUB+SUB-1)
                    nc.scalar.copy(
                        w0[:, i1 * SUB:(i1 + 1) * SUB, :].rearrange(
                            "p c r -> p (c r)"),
                        pg[:],
                    )

                # FWHT butterflies over i1 across chunks; ping-pong w0 <-> xin
                src, dst = w0, xin
                for st in range(N_STAGES):
                    h = 1 << st              # i1-unit distance
                    nb = A // (2 * h)
                    ch = SUB * h             # chunks per half-block
                    sv = src[:].rearrange("p (nb two ch) r -> p nb two ch r",
                                          nb=nb, two=2)
                    dv = dst[:].rearrange("p (nb two ch) r -> p nb two ch r",
                                          nb=nb, two=2)
                    for (eng, j0, j1) in (("v", 0, RD), ("g", RD, R)):
                        if j0 >= j1:
                            continue
                        top = sv[:, :, 0, :, j0:j1]
                        bot = sv[:, :, 1, :, j0:j1]
                        oa = dv[:, :, 0, :, j0:j1]
                        ob = dv[:, :, 1, :, j0:j1]
                        if eng == "v":
                            nc.vector.tensor_add(oa, top, bot)
                            nc.vector.tensor_sub(ob, top, bot)
                        else:
                            nc.gpsimd.tensor_add(oa, top, bot)
                            nc.gpsimd.tensor_sub(ob, top, bot)
                    src, dst = dst, src

                # final result is in `src` after the last swap
                nc.sync.dma_start(yT_v[:, :, r0:r0 + R], src[:])

    nc.compile()
    return nc


_prog = None


def _get_prog():
    global _prog
    if _prog is None:
        _prog = _build()
    return _prog


def _run(xT, Hf, trace=False):
    nc = _get_prog()
    in_maps = [
        {"xT": np.ascontiguousarray(xT[:, c * RC:(c + 1) * RC]), "Hf": Hf}
        for c in range(N_CORES)
    ]
    res = run_bass_kernel_spmd(nc, in_maps, core_ids=list(range(N_CORES)),
                               trace=trace)
    return res


def kernel(x, H):
    x = np.asarray(x)
    H = np.asarray(H)
    xT = np.ascontiguousarray(x.reshape(ROWS, N).T)          # [N, ROWS]
    Hf = np.ascontiguousarray(H[:B, :B])                      # = H_B / 64
    res = _run(xT, Hf)
    y = np.empty((ROWS, N), dtype=np.float32)
    for c in range(N_CORES):
        y[c * RC:(c + 1) * RC, :] = res.results[c]["yT"].T
    return y.reshape(4, 4096, N)


# revision 8
# speedup vs baseline: 1.0352x; 1.0352x over previous
"""Trainium2 Bass kernel for BotanHadamardTransform: y = x @ H, with
x [4, 4096, 4096] f32 and H [4096, 4096] f32 the normalized Sylvester
Hadamard matrix H_4096 / 64.

Algorithm: Sylvester Hadamard matrices factor as Kronecker products,
H_4096 = H_A (x) H_B with A*B = 4096. For a row vector v (len 4096),
v @ H_4096 = FWHT_A applied across the A axis of (v.reshape(A, B) @ H_B).
This reduces per-row work from O(n^2) to O(n*(B + log2 A)).

Mapping to hardware (per core, 1/8 of the 16384 rows = 2048 rows):
  - host pre-transposes x so the device sees xT [4096, 2048] with the
    contraction dim on partitions (natural matmul layout, no on-device
    transposes)
  - PE contracts the low B=512 of each k-index against Hf = H[0:512,0:512]
    (which equals H_512/64 exactly) as fp32r matmuls, N=512 moving columns
  - the high A=8 factor is a 3-stage FWHT butterfly across 128-partition
    chunks; stage 1 runs on VectorE directly out of PSUM (doubling as the
    eviction); stages 2-3 are split between VectorE and GpSimdE by chunk
    range with long contiguous inner rows
  - output is written transposed (yT [4096, 2048]); host transposes back
"""
import os
import sys

sys.path.insert(0, "/opt/trn_rl_repo")

import numpy as np

import concourse.bass as bass  # noqa: F401
import concourse.tile as tile
from concourse import bacc, mybir
from concourse.bass_utils import run_bass_kernel_spmd

N_CORES = 8
N = 4096            # hidden dim
ROWS = 4 * 4096     # total rows
RC = ROWS // N_CORES  # rows (columns of xT) per core = 2048

# tunables (env-overridable for experiments)
B = int(os.environ.get("BOTAN_B", "512"))        # PE-contracted factor
R = int(os.environ.get("BOTAN_R", "512"))        # moving columns per r-tile
MM_DTYPE = os.environ.get("BOTAN_DT", "f32r")    # f32 | f32r
# GpSimd chunk share numerator (of 16ths) for butterfly stages 2+
GP16 = int(os.environ.get("BOTAN_GP16", "7"))

A = N // B               # butterfly factor (8)
SUB = B // 128           # accumulating matmuls per output chunk (4)
NCH = N // 128           # 32 chunks of 128 partitions
N_STAGES = A.bit_length() - 1  # 3


def _build():
    nc = bacc.Bacc("TRN2", target_bir_lowering=False, debug=False,
                   num_devices=N_CORES)
    xT_ap = nc.dram_tensor("xT", [N, RC], mybir.dt.float32,
                           kind="ExternalInput").ap()
    hf_ap = nc.dram_tensor("Hf", [B, B], mybir.dt.float32,
                           kind="ExternalInput").ap()
    yT_ap = nc.dram_tensor("yT", [N, RC], mybir.dt.float32,
                           kind="ExternalOutput").ap()

    f32 = mybir.dt.float32
    f32r = mybir.dt.float32r
    mm_dt = f32r if MM_DTYPE == "f32r" else f32

    xT_v = xT_ap.rearrange("(c p) r -> p c r", p=128)   # [128, NCH, RC]
    yT_v = yT_ap.rearrange("(c p) r -> p c r", p=128)

    n_rt = RC // R
    npair = A // 2  # 4 i1-pairs per r-tile

    with tile.TileContext(nc) as tc:
        with (
            tc.tile_pool(name="hfp", bufs=1) as hfp,
            tc.tile_pool(name="xin", bufs=1) as xinp,
            tc.tile_pool(name="xr", bufs=2) as xrp,
            tc.tile_pool(name="w0", bufs=1) as w0p,
            tc.tile_pool(name="ev", bufs=2) as evp,
            tc.tile_pool(name="ps", bufs=2, space="PSUM") as psp,
        ):
            # stationary: Hf [B, B] stored as SUB row-blocks side by side:
            # hf[p, s*B + col] = Hf[s*128 + p, col]
            hf_f32 = hfp.tile([128, SUB * B], f32, tag="hf32")
            for s in range(SUB):
                nc.sync.dma_start(hf_f32[:, s * B:(s + 1) * B],
                                  hf_ap[s * 128:(s + 1) * 128, :])
            if mm_dt == f32r:
                hf_mm = hfp.tile([128, SUB * B], f32r, tag="hfr")
                nc.scalar.copy(hf_mm[:], hf_f32[:])
            else:
                hf_mm = hf_f32

            def hf_block(s, q):
                # lhsT block [k=128 (i2 sub s), m=128 (j2 sub q)]
                return hf_mm[:, s * B + q * 128: s * B + q * 128 + 128]

            for it in range(n_rt):
                r0 = it * R
                xin = xinp.tile([128, NCH, R], f32, tag="xin")
                w0 = w0p.tile([128, NCH, R], f32, tag="w0")

                # DMA in + transform + fused stage-1, per i1-pair block;
                # PSUM work is split into q-halves so two blocks can be in
                # flight (4 banks each) and PE never stalls on stage-1
                QH = 2  # q-values per half-block
                for m in range(npair):
                    ch0 = m * 2 * SUB  # first chunk of the pair block
                    nc.sync.dma_start(
                        xin[:, ch0:ch0 + 2 * SUB, :],
                        xT_v[:, ch0:ch0 + 2 * SUB, r0:r0 + R])
                    if mm_dt == f32r:
                        xg = xrp.tile([128, 2 * SUB, R], f32r, tag="xr")
                        nc.scalar.copy(xg[:], xin[:, ch0:ch0 + 2 * SUB, :])
                    else:
                        xg = xin[:, ch0:ch0 + 2 * SUB, :]

                    for qh in range(SUB // QH):
                        pg = [psp.tile([128, QH * R], f32, tag=f"pg{j}",
                                       name=f"pg{j}_{it}_{m}_{qh}")
                              for j in range(2)]
                        for qq in range(QH):
                            q = qh * QH + qq
                            for s in range(SUB):
                                for j in range(2):
                                    nc.tensor.matmul(
                                        pg[j][:, qq * R:(qq + 1) * R],
                                        hf_block(s, q),
                                        xg[:, j * SUB + s, :],
                                        start=(s == 0),
                                        stop=(s == SUB - 1),
                                    )
                        # stage-1 butterfly fused with eviction: ScalarE
                        # evicts pg0 to SBUF scratch, then VectorE adds/subs
                        # against pg1 still in PSUM (DVE has 1 PSUM port)
                        ca = ch0 + qh * QH
                        cb = ch0 + SUB + qh * QH
                        ev = evp.tile([128, QH * R], f32, tag="ev",
                                      name=f"ev_{it}_{m}_{qh}")
                        nc.scalar.copy(ev[:], pg[0][:])
                        oa = w0[:, ca:ca + QH, :].rearrange("p c r -> p (c r)")
                        ob = w0[:, cb:cb + QH, :].rearrange("p c r -> p (c r)")
                        nc.vector.tensor_add(oa, ev[:], pg[1][:])
                        nc.vector.tensor_sub(ob, ev[:], pg[1][:])

                # stages 2..: ping-pong w0 -> xin -> w0 ...
                src, dst = w0, xin
                for st in range(1, N_STAGES):
                    h = 1 << st              # i1-unit distance
                    nb = A // (2 * h)
                    ch = SUB * h             # chunks per half-block
                    sv = src[:].rearrange("p (nb two ch) r -> p nb two ch r",
                                          nb=nb, two=2)
                    dv = dst[:].rearrange("p (nb two ch) r -> p nb two ch r",
                                          nb=nb, two=2)
                    # split by chunk range: GpSimd takes the tail GP16/16ths
                    c_gp = (ch * (16 - GP16) + 15) // 16
                    for (eng, c0, c1) in (("v", 0, c_gp), ("g", c_gp, ch)):
                        if c0 >= c1:
                            continue
                        top = sv[:, :, 0, c0:c1, :]
                        bot = sv[:, :, 1, c0:c1, :]
                        oa = dv[:, :, 0, c0:c1, :]
                        ob = dv[:, :, 1, c0:c1, :]
                        if eng == "v":
                            nc.vector.tensor_add(oa, top, bot)
                            nc.vector.tensor_sub(ob, top, bot)
                        else:
                            nc.gpsimd.tensor_add(oa, top, bot)
                            nc.gpsimd.tensor_sub(ob, top, bot)
                    src, dst = dst, src

                # final result is in `src` after the last swap;
                # DMA out per quarter so buffers free progressively
                for m in range(4):
                    ch0 = m * (NCH // 4)
                    nc.sync.dma_start(
                        yT_v[:, ch0:ch0 + NCH // 4, r0:r0 + R],
                        src[:, ch0:ch0 + NCH // 4, :])

    nc.compile()
    return nc


_prog = None


def _get_prog():
    global _prog
    if _prog is None:
        _prog = _build()
    return _prog


def _run(xT, Hf, trace=False):
    nc = _get_prog()
    in_maps = [
        {"xT": np.ascontiguousarray(xT[:, c * RC:(c + 1) * RC]), "Hf": Hf}
        for c in range(N_CORES)
    ]
    res = run_bass_kernel_spmd(nc, in_maps, core_ids=list(range(N_CORES)),
                               trace=trace)
    return res


def kernel(x, H):
    x = np.asarray(x)
    H = np.asarray(H)
    xT = np.ascontiguousarray(x.reshape(ROWS, N).T)          # [N, ROWS]
    Hf = np.ascontiguousarray(H[:B, :B])                      # = H_B / 64
    res = _run(xT, Hf)
    y = np.empty((ROWS, N), dtype=np.float32)
    for c in range(N_CORES):
        y[c * RC:(c + 1) * RC, :] = res.results[c]["yT"].T
    return y.reshape(4, 4096, N)


# revision 9
# speedup vs baseline: 1.1469x; 1.1079x over previous
"""Trainium2 Bass kernel for BotanHadamardTransform: y = x @ H, with
x [4, 4096, 4096] f32 and H [4096, 4096] f32 the normalized Sylvester
Hadamard matrix H_4096 / 64.

Algorithm: Sylvester Hadamard matrices factor as Kronecker products,
H_4096 = H_A (x) H_B with A*B = 4096. For a row vector v (len 4096),
v @ H_4096 = FWHT_A applied across the A axis of (v.reshape(A, B) @ H_B).
This reduces per-row work from O(n^2) to O(n*(B + log2 A)).

Mapping to hardware (per core, 1/8 of the 16384 rows = 2048 rows):
  - host pre-transposes x so the device sees xT [4096, 2048] with the
    contraction dim on partitions (natural matmul layout, no on-device
    transposes)
  - PE contracts the low B=512 of each k-index against Hf = H[0:512,0:512]
    (which equals H_512/64 exactly) as fp32r matmuls, N=512 moving columns
  - the high A=8 factor is a 3-stage FWHT butterfly across 128-partition
    chunks; stage 1 runs fused with PSUM eviction (ScalarE evicts one
    accumulator, VectorE adds/subs against the other still in PSUM);
    stages 2-3 are whole-block VectorE ops with fully contiguous access
    patterns, with an optional thin GpSimd chunk slice
  - output is written transposed (yT [4096, 2048]); host transposes back

Buffer scheme per r-tile (R=512 moving columns, 4 r-tiles per core):
  G1 blocks (xinb, f32 [128,8,512]): DMA-in dest; dead after rounding;
     reused as stage-1 output (the butterfly ping); s2 reads them.
  xr blocks (f32r): rounded matmul input; dead after matmuls; slots
     reused for stage-2 outputs (f32 bitcast view); s3 reads those.
  s3 writes fresh G1-pool blocks; DMA-out drains them.
"""
import os
import sys

sys.path.insert(0, "/opt/trn_rl_repo")

import numpy as np

import concourse.bass as bass  # noqa: F401
import concourse.tile as tile
from concourse import bacc, mybir
from concourse.bass_utils import run_bass_kernel_spmd

N_CORES = 8
N = 4096            # hidden dim
ROWS = 4 * 4096     # total rows
RC = ROWS // N_CORES  # rows (columns of xT) per core = 2048

# tunables (env-overridable for experiments)
B = int(os.environ.get("BOTAN_B", "512"))        # PE-contracted factor
R = int(os.environ.get("BOTAN_R", "512"))        # moving columns per r-tile
MM_DTYPE = os.environ.get("BOTAN_DT", "f32r")    # f32 | f32r
# GpSimd chunk share (in 16ths) of stage-2+ butterfly ops
GP16 = int(os.environ.get("BOTAN_GP16", "2"))
XINB_BUFS = int(os.environ.get("BOTAN_XINB", "5"))
XR_BUFS = int(os.environ.get("BOTAN_XRB", "5"))

A = N // B               # butterfly factor (8)
SUB = B // 128           # accumulating matmuls per output chunk (4)
NCH = N // 128           # 32 chunks of 128 partitions
BCH = 2 * SUB            # chunks per pair-block (8)
NPAIR = A // 2           # pair blocks (4)
QH = 2                   # q-values per PSUM half-block


def _build():
    nc = bacc.Bacc("TRN2", target_bir_lowering=False, debug=False,
                   num_devices=N_CORES)
    xT_ap = nc.dram_tensor("xT", [N, RC], mybir.dt.float32,
                           kind="ExternalInput").ap()
    hf_ap = nc.dram_tensor("Hf", [B, B], mybir.dt.float32,
                           kind="ExternalInput").ap()
    yT_ap = nc.dram_tensor("yT", [N, RC], mybir.dt.float32,
                           kind="ExternalOutput").ap()

    f32 = mybir.dt.float32
    f32r = mybir.dt.float32r

    xT_v = xT_ap.rearrange("(c p) r -> p c r", p=128)   # [128, NCH, RC]
    yT_v = yT_ap.rearrange("(c p) r -> p c r", p=128)

    n_rt = RC // R

    with tile.TileContext(nc) as tc:
        with (
            tc.tile_pool(name="hfp", bufs=1) as hfp,
            tc.tile_pool(name="xinb", bufs=XINB_BUFS) as xinbp,
            tc.tile_pool(name="xr", bufs=XR_BUFS) as xrp,
            tc.tile_pool(name="ev", bufs=2) as evp,
            tc.tile_pool(name="ps", bufs=2, space="PSUM") as psp,
        ):
            # stationary Hf: stage f32 via an xr-pool slot, round to f32r.
            # layout: hf[p, s*B + col] = Hf[s*128 + p, col]
            hf_st = xrp.tile([128, SUB * B], f32, tag="xr", name="hf_stage")
            for s in range(SUB):
                nc.sync.dma_start(hf_st[:, s * B:(s + 1) * B],
                                  hf_ap[s * 128:(s + 1) * 128, :])
            hf_mm = hfp.tile([128, SUB * B], f32r, tag="hfr")
            nc.scalar.copy(hf_mm[:], hf_st[:])

            def hf_block(s, q):
                # lhsT block [k=128 (i2 sub s), m=128 (j2 sub q)]
                return hf_mm[:, s * B + q * 128: s * B + q * 128 + 128]

            def bf_pair(dst_add, dst_sub, src0, src1, tag):
                """dst_add = src0+src1, dst_sub = src0-src1 on [128,BCH,R]
                tiles (flattened, fully contiguous); GpSimd takes the tail
                GP16/16 of chunks."""
                c_gp = (BCH * (16 - GP16)) // 16
                for (eng, c0, c1) in (("v", 0, c_gp), ("g", c_gp, BCH)):
                    if c0 >= c1:
                        continue
                    sl = lambda t: t[:, c0:c1, :].rearrange("p c r -> p (c r)")
                    if eng == "v":
                        nc.vector.tensor_add(sl(dst_add), sl(src0), sl(src1))
                        nc.vector.tensor_sub(sl(dst_sub), sl(src0), sl(src1))
                    else:
                        nc.gpsimd.tensor_add(sl(dst_add), sl(src0), sl(src1))
                        nc.gpsimd.tensor_sub(sl(dst_sub), sl(src0), sl(src1))

            for it in range(n_rt):
                r0 = it * R
                g1 = []   # pair-block tiles: input -> stage-1 output
                for m in range(NPAIR):
                    ch0 = m * BCH
                    xb = xinbp.tile([128, BCH, R], f32, tag="xinb",
                                    name=f"xb_{it}_{m}")
                    g1.append(xb)
                    nc.sync.dma_start(xb[:],
                                      xT_v[:, ch0:ch0 + BCH, r0:r0 + R])
                    # rounding pass f32 -> f32r (ScalarE); xb is dead after
                    # this and becomes the stage-1 destination
                    xg = xrp.tile([128, BCH, R], f32r, tag="xr",
                                  name=f"xg_{it}_{m}")
                    nc.scalar.copy(xg[:], xb[:])

                    for qh in range(SUB // QH):
                        pg = [psp.tile([128, QH * R], f32, tag=f"pg{j}",
                                       name=f"pg{j}_{it}_{m}_{qh}")
                              for j in range(2)]
                        for qq in range(QH):
                            q = qh * QH + qq
                            for s in range(SUB):
                                for j in range(2):
                                    nc.tensor.matmul(
                                        pg[j][:, qq * R:(qq + 1) * R],
                                        hf_block(s, q),
                                        xg[:, j * SUB + s, :],
                                        start=(s == 0),
                                        stop=(s == SUB - 1),
                                    )
                        # stage-1 butterfly fused with eviction: ScalarE
                        # evicts pg0, VectorE adds/subs against pg1 in PSUM.
                        # outputs go into the dead input block xb:
                        # local chunks [qh*QH, qh*QH+QH) (i1=2m half) and
                        # [SUB+qh*QH, ...) (i1=2m+1 half)
                        ev = evp.tile([128, QH * R], f32, tag="ev",
                                      name=f"ev_{it}_{m}_{qh}")
                        nc.scalar.copy(ev[:], pg[0][:])
                        ca = qh * QH
                        cb = SUB + qh * QH
                        oa = xb[:, ca:ca + QH, :].rearrange("p c r -> p (c r)")
                        ob = xb[:, cb:cb + QH, :].rearrange("p c r -> p (c r)")
                        nc.vector.tensor_add(oa, ev[:], pg[1][:])
                        nc.vector.tensor_sub(ob, ev[:], pg[1][:])

                # stage 2: block-pair adds (xr slots freed by the matmuls
                # become the f32 destinations via fresh pool tiles)
                g2 = [xrp.tile([128, BCH, R], f32, tag="xr",
                               name=f"g2_{it}_{i}") for i in range(4)]
                bf_pair(g2[0], g2[1], g1[0], g1[1], "s2a")
                bf_pair(g2[2], g2[3], g1[2], g1[3], "s2b")

                # stage 3: outputs in final chunk order, fresh g1-pool tiles
                g3 = [xinbp.tile([128, BCH, R], f32, tag="xinb",
                                 name=f"g3_{it}_{i}") for i in range(4)]
                bf_pair(g3[0], g3[2], g2[0], g2[2], "s3a")
                bf_pair(g3[1], g3[3], g2[1], g2[3], "s3b")

                for i in range(4):
                    nc.sync.dma_start(
                        yT_v[:, i * BCH:(i + 1) * BCH, r0:r0 + R], g3[i][:])

    nc.compile()
    return nc


_prog = None


def _get_prog():
    global _prog
    if _prog is None:
        _prog = _build()
    return _prog


def _run(xT, Hf, trace=False):
    nc = _get_prog()
    in_maps = [
        {"xT": np.ascontiguousarray(xT[:, c * RC:(c + 1) * RC]), "Hf": Hf}
        for c in range(N_CORES)
    ]
    res = run_bass_kernel_spmd(nc, in_maps, core_ids=list(range(N_CORES)),
                               trace=trace)
    return res


def kernel(x, H):
    x = np.asarray(x)
    H = np.asarray(H)
    xT = np.ascontiguousarray(x.reshape(ROWS, N).T)          # [N, ROWS]
    Hf = np.ascontiguousarray(H[:B, :B])                      # = H_B / 64
    res = _run(xT, Hf)
    y = np.empty((ROWS, N), dtype=np.float32)
    for c in range(N_CORES):
        y[c * RC:(c + 1) * RC, :] = res.results[c]["yT"].T
    return y.reshape(4, 4096, N)
